# revision 1
# baseline (speedup 1.0000x reference)
"""Causal self-attention (single head) on 8 TRN2 NeuronCores.

Sharding: data-parallel over batch (4) x query-interleave (2).
Core c handles batch b = c//2 and 8 query blocks of 128 chosen so that
the two cores of a batch have equal causal work:
  j=0 -> blocks [0,3,4,7,8,11,12,15],  j=1 -> blocks [1,2,5,6,9,10,13,14]
Slot t of each core processes its query block against the first
256*(t+1) keys (a superset of the causal requirement for both cores'
blocks in that slot), with the exact causal mask applied from per-core
query-position data.  All cores run one identical SPMD program; only
input data differs.

Compute dtype: bf16 matmuls (f32 PSUM accumulate), f32 softmax stats.
"""

from contextlib import ExitStack

import numpy as np
import ml_dtypes

B, S, D = 4, 2048, 1024
P = 128
ND = D // P  # 8 d (contraction) chunks
NE = D // P  # 8 output-feature chunks
NSK = S // P  # 16 key chunks
NQB = 8  # query blocks per core
SQH = NQB * P  # 1024 queries per core
J_BLOCKS = (
    [0, 3, 4, 7, 8, 11, 12, 15],
    [1, 2, 5, 6, 9, 10, 13, 14],
)
COVS = [256 * (t + 1) for t in range(NQB)]  # key coverage per slot
SCALE = 1.0 / np.sqrt(np.float32(D))  # 1/32
NEG_BIG = -1.0e30
CPAK = NQB + 2 * NE + S + 2 * D  # packed f32 consts width

_NC = None


def _score_tiles(cov):
    """(offset, width) score tiles of <=512 columns covering [0, cov)."""
    tiles = [(off, 512) for off in range(0, cov - cov % 512, 512)]
    if cov % 512:
        tiles.append((cov - cov % 512, cov % 512))
    return tiles


def _emit(nc, tc, dr, out_d):
    import concourse.bass as bass  # noqa: F401
    from concourse import mybir

    BF = mybir.dt.bfloat16
    F32 = mybir.dt.float32
    AF = mybir.ActivationFunctionType
    Alu = mybir.AluOpType
    X = mybir.AxisListType.X

    with ExitStack() as ctx:
        const = ctx.enter_context(tc.tile_pool(name="const", bufs=1))
        # packed f32 consts: tiny slice first (needed by the first
        # evictions), bulky slice (kpos/bv/bo) loaded later off the
        # critical path
        cpak = const.tile([P, CPAK], F32)
        nc.sync.dma_start(cpak[:, : NQB + 2 * NE], dr["cpak"][:, : NQB + 2 * NE])
        qpos = cpak[:, 0:NQB]
        bq = cpak[:, NQB : NQB + NE]
        bk = cpak[:, NQB + NE : NQB + 2 * NE]
        kpos = cpak[:, NQB + 2 * NE : NQB + 2 * NE + S]
        bv = cpak[:, NQB + 2 * NE + S : NQB + 2 * NE + S + D]
        bo = cpak[:, NQB + 2 * NE + S + D : NQB + 2 * NE + S + 2 * D]
        ident = const.tile([P, P], BF)
        nc.sync.dma_start(ident[:], dr["ident"])
        # persistent activation storage
        qt_pool = ctx.enter_context(tc.tile_pool(name="qt", bufs=NE))
        v_pool = ctx.enter_context(tc.tile_pool(name="v", bufs=NSK))
        xt_pool = ctx.enter_context(tc.tile_pool(name="xt", bufs=ND))
        QT, V = [], []

        # ---------------- phase A: projections ----------------
        # Keys are host-permuted so this core's query block t sits at
        # columns [256t, 256t+128) of xT — Q is projected straight from
        # xT slices (no separate x_q input).  Weight chunks arrive one
        # batched DMA per e-group (8 chunks) to keep the per-trigger
        # sync-engine cost (~0.7us each) off the critical path.
        with ExitStack() as actx:
            wvt_pool = actx.enter_context(tc.tile_pool(name="wvt", bufs=ND))
            wstr_pool = actx.enter_context(tc.tile_pool(name="wstr", bufs=4))
            psp = actx.enter_context(tc.tile_pool(name="psp", bufs=8, space="PSUM"))

            XT = [None] * ND
            WV = []

            def load_wgroup(key, e, tag, bufs=None, eng=None):
                wg = wstr_pool.tile(
                    [P, ND * P], BF, tag=tag, name=tag, bufs=bufs
                )
                (eng or nc.scalar).dma_start(wg[:], dr[key][e])
                return wg

            # x stream split across both rings, interleaved with the wq
            # groups so neither the first matmul nor the d-loop starves:
            #   sync ring:   xT[0], xT[2], xT[4], xT[6]
            #   scalar ring: wq[0], xT[1], wq[1], xT[3], ... wq[4..7]
            for d in (0, 4):
                xt = xt_pool.tile([P, S], BF, name="xt")
                nc.sync.dma_start(xt[:], dr["xT"][d])
                XT[d] = xt
            for d in (1, 2, 6):
                xt = xt_pool.tile([P, S], BF, name="xt")
                nc.gpsimd.dma_start(xt[:], dr["xT"][d])
                XT[d] = xt
            WQG = []
            for e in range(4):
                WQG.append(load_wgroup("wqT", e, "wq", bufs=NE))
                if e > 0:
                    d = 2 * e + 1
                    xt = xt_pool.tile([P, S], BF, name="xt")
                    nc.scalar.dma_start(xt[:], dr["xT"][d])
                    XT[d] = xt
            for e in range(4, NE):
                # sync ring is idle after the xT evens — balance the load
                WQG.append(load_wgroup("wqT", e, "wq", bufs=NE, eng=nc.sync))

            # QT[e] = (Wq @ x_q^T)[e-chunk] + bq  -> [128 e, 1024 q] bf16
            for e in range(NE):
                qts = qt_pool.tile([P, SQH], BF)
                wg = WQG[e]
                pss = [psp.tile([P, 512], F32, tag="ps", name="ps") for _ in range(2)]
                for d in range(ND):
                    for nt in range(2):
                        # strided rhs: first 128 of each 256-col slot pair
                        # (this core's queries) -> N=512 in one matmul
                        rhs = XT[d][
                            :, nt * 1024 : (nt + 1) * 1024
                        ].rearrange("p (t c) -> p t c", t=4)[:, :, 0:P]
                        nc.tensor.matmul(
                            pss[nt][:],
                            wg[:, d * P : (d + 1) * P],
                            rhs,
                            start=(d == 0),
                            stop=(d == ND - 1),
                        )
                for nt in range(2):
                    nc.scalar.activation(
                        qts[:, nt * 512 : (nt + 1) * 512],
                        pss[nt][:],
                        AF.Identity,
                        bias=bq[:, e : e + 1],
                        scale=1.0,
                    )
                QT.append(qts)

            for d in range(ND):
                wv = wvt_pool.tile([P, D], BF)
                nc.sync.dma_start(wv[:], dr["wvT"][d])
                WV.append(wv)
            nc.sync.dma_start(
                cpak[:, NQB + 2 * NE :], dr["cpak"][:, NQB + 2 * NE :]
            )

            # V[s] = (x @ Wv^T)[s-chunk] + bv -> [128 s, 1024 e] bf16
            for s in range(NSK):
                vs = v_pool.tile([P, D], BF)
                pss = [psp.tile([P, 512], F32, tag="ps", name="ps") for _ in range(2)]
                for d in range(ND):
                    for nt in range(2):
                        nc.tensor.matmul(
                            pss[nt][:],
                            XT[d][:, s * P : (s + 1) * P],
                            WV[d][:, nt * 512 : (nt + 1) * 512],
                            start=(d == 0),
                            stop=(d == ND - 1),
                        )
                for nt in range(2):
                    nc.scalar.activation(
                        vs[:, nt * 512 : (nt + 1) * 512],
                        pss[nt][:],
                        AF.Copy,
                    )
                V.append(vs)

        # ---------------- phase B: attention + output projection ----------------
        # 3-stage software pipeline over query blocks (big blocks first):
        #   S(t): score matmuls + mask + softmax + DMA-transpose of weights
        #   A(t): attended matmuls + rinv-scaled evict + DMA-transpose
        #   O(t): output projection + bias + store
        # PE program order S(i);A(i-1);O(i-2) keeps the PE dense while
        # softmax/DMA latencies of a block hide under the next block's
        # score matmuls.
        with ExitStack() as bctx:
            sp = bctx.enter_context(tc.tile_pool(name="s_sb", bufs=2))
            wp = bctx.enter_context(tc.tile_pool(name="w_sb", bufs=2))
            wtp = bctx.enter_context(tc.tile_pool(name="wt_sb", bufs=3))
            outp = bctx.enter_context(tc.tile_pool(name="out_sb", bufs=2))
            stat = bctx.enter_context(tc.tile_pool(name="stat", bufs=3))
            ps_s = bctx.enter_context(tc.tile_pool(name="ps_s", bufs=3, space="PSUM"))
            ps_t = bctx.enter_context(tc.tile_pool(name="ps_t", bufs=2, space="PSUM"))
            ps_a = bctx.enter_context(tc.tile_pool(name="ps_a", bufs=3, space="PSUM"))

            def emit_scores(t):
                cov = COVS[t]
                s_sb = sp.tile([P, cov], F32, tag="s")
                # mask term: (k > q) * -1e30, written into s_sb
                nc.vector.tensor_scalar(
                    s_sb[:],
                    kpos[:, :cov],
                    qpos[:, t : t + 1],
                    NEG_BIG,
                    op0=Alu.is_gt,
                    op1=Alu.mult,
                )
                for off, wdt in _score_tiles(cov):
                    ps = ps_s.tile([P, wdt], F32, tag="ps_s")
                    for e in range(NE):
                        nc.tensor.matmul(
                            ps[:],
                            QT[e][:, t * P : (t + 1) * P],
                            XT[e][:, off : off + wdt],
                            start=(e == 0),
                            stop=(e == NE - 1),
                        )
                    nc.vector.tensor_tensor(
                        s_sb[:, off : off + wdt],
                        ps[:],
                        s_sb[:, off : off + wdt],
                        op=Alu.add,
                    )
                negm = stat.tile([P, 1], F32, tag="negm")
                nc.vector.tensor_reduce(
                    negm[:], s_sb[:], axis=X, op=Alu.max, negate=True
                )
                negm32 = stat.tile([P, 1], F32, tag="negm32")
                nc.vector.tensor_scalar_mul(negm32[:], negm[:], float(SCALE))
                w_sb = wp.tile([P, cov], BF, tag="w")
                lsum = stat.tile([P, 1], F32, tag="lsum")
                nc.scalar.activation(
                    w_sb[:],
                    s_sb[:],
                    AF.Exp,
                    bias=negm32[:],
                    scale=float(SCALE),
                    accum_out=lsum[:],
                )
                rinv = stat.tile([P, 1], F32, tag="rinv")
                nc.vector.reciprocal(rinv[:], lsum[:])
                # weight transposes on PE (matmul transpose mode)
                K = cov // P
                wT = wtp.tile([P, cov], BF, tag="wt")
                for k in range(K):
                    pt = ps_t.tile([P, P], BF, tag="pt")
                    nc.tensor.transpose(pt[:], w_sb[:, k * P : (k + 1) * P], ident[:])
                    nc.vector.tensor_copy(wT[:, k * P : (k + 1) * P], pt[:])
                return {"t": t, "wT": wT, "rinv": rinv}

            def emit_attend(st):
                t = st["t"]
                cov = COVS[t]
                K = cov // P
                wT, rinv = st["wT"], st["rinv"]
                outsb = outp.tile([P, D], F32, tag="o")
                for nt in range(2):
                    pa = ps_a.tile([P, 512], F32, tag="pa")
                    for k in range(K):
                        nc.tensor.matmul(
                            pa[:],
                            wT[:, k * P : (k + 1) * P],
                            V[k][:, nt * 512 : (nt + 1) * 512],
                            start=(k == 0),
                            stop=(k == K - 1),
                        )
                    # out = psum * rinv (softmax normalize) then + bvo
                    nc.scalar.activation(
                        outsb[:, nt * 512 : (nt + 1) * 512],
                        pa[:],
                        AF.Copy,
                        bias=0.0,
                        scale=rinv[:],
                    )
                    nc.vector.tensor_tensor(
                        outsb[:, nt * 512 : (nt + 1) * 512],
                        outsb[:, nt * 512 : (nt + 1) * 512],
                        bo[:, nt * 512 : (nt + 1) * 512],
                        op=Alu.add,
                    )
                    nc.sync.dma_start(
                        out_d[t][:, nt * 512 : (nt + 1) * 512],
                        outsb[:, nt * 512 : (nt + 1) * 512],
                    )

            order = list(range(NQB - 1, -1, -1))  # big blocks first
            states = []
            for i, t in enumerate(order):
                states.append(emit_scores(t))
                if i >= 1:
                    emit_attend(states[i - 1])
            emit_attend(states[-1])


def build_nc():
    """Build + compile the SPMD Bass program (cached)."""
    global _NC
    if _NC is not None:
        return _NC
    from concourse import bacc, mybir
    import concourse.tile as tile

    BF = mybir.dt.bfloat16
    F32 = mybir.dt.float32

    nc = bacc.Bacc(
        "TRN2", target_bir_lowering=False, debug=False, enable_asserts=False
    )
    dr = {}

    def din(name, shape, dt):
        dr[name] = nc.dram_tensor(name, shape, dt, kind="ExternalInput").ap()

    din("xT", (ND, P, S), BF)
    din("wqT", (NE, P, ND * P), BF)
    din("wvT", (ND, P, D), BF)
    din("ident", (P, P), BF)
    din("cpak", (P, CPAK), F32)
    out_d = nc.dram_tensor("out_c", (NQB, P, D), F32, kind="ExternalOutput").ap()

    with tile.TileContext(nc) as tc:
        _emit(nc, tc, dr, out_d)
    nc.compile()
    _NC = nc
    return nc


def make_in_maps(x, Wq, bq, Wk, bk, Wv, bv, Wo, bo):
    """Host-side sharding: per-core input dicts (bf16 compute operands)."""
    bf16 = ml_dtypes.bfloat16
    f32 = np.float32

    def chunkg(WT):  # [1024,1024] -> (NE,128,ND*128): [e][p][d*128+c]
        return (
            np.ascontiguousarray(
                WT.reshape(ND, P, NE, P).transpose(2, 1, 0, 3)
            ).reshape(NE, P, ND * P)
        ).astype(bf16)

    # host-fused weights (f32 GEMMs, exact up to fp32):
    #   scores = (x Wq^T)(x Wk^T)^T = x (Wq^T Wk) x^T       -> Wqk
    #   out    = softmax(..) (x Wv^T) Wo^T = softmax(..) x (Wo Wv)^T
    # so K and the output projection never materialize on-chip.
    # Requires bq = bk = 0 (guaranteed by the problem spec).
    Wqk = Wq.T.astype(np.float32) @ Wk.astype(np.float32)  # [d1, d2]
    Wvo = Wo.astype(np.float32) @ Wv.astype(np.float32)  # [e, d]
    wq_c = chunkg(Wqk)
    wv_c = np.ascontiguousarray(Wvo.T).reshape(ND, P, D).astype(bf16)
    bvo = Wo.astype(np.float32) @ bv.astype(np.float32) + bo.astype(np.float32)
    bq_t = np.ascontiguousarray(bq.reshape(NE, P).T).astype(f32)
    bk_t = np.ascontiguousarray(bk.reshape(NE, P).T).astype(f32)
    bv_b = np.zeros((P, D), f32)  # bv folded into bvo
    bo_b = np.broadcast_to(bvo, (P, D))
    ident = np.eye(P, dtype=bf16)

    in_maps = []
    for c in range(8):
        b, j = c // 2, c % 2
        blocks = J_BLOCKS[j]
        other = J_BLOCKS[1 - j]
        # key permutation: slot t holds [my block t | peer block t], so
        # this core's queries are columns [256t, 256t+128) and the first
        # 256(t+1) columns cover every true key <= any query in slot t
        perm = np.concatenate(
            [
                np.r_[P * blocks[t] : P * (blocks[t] + 1),
                      P * other[t] : P * (other[t] + 1)]
                for t in range(NQB)
            ]
        )
        xTb = np.ascontiguousarray(x[b].T[:, perm])  # [D, S] permuted keys
        qpos = (np.array(blocks, dtype=f32) * P)[None, :] + np.arange(
            P, dtype=f32
        )[:, None]
        kpos = np.broadcast_to(perm.astype(f32), (P, S))
        cpak = np.concatenate([qpos, bq_t, bk_t, kpos, bv_b, bo_b], axis=1)
        assert cpak.shape == (P, CPAK)
        in_maps.append(
            {
                "xT": xTb.reshape(ND, P, S).astype(bf16),
                "wqT": wq_c,
                "wvT": wv_c,
                "cpak": np.ascontiguousarray(cpak.astype(f32)),
                "ident": ident,
            }
        )
    return in_maps


def assemble_out(results):
    out = np.empty((B, S, D), dtype=np.float32)
    for c in range(8):
        b, j = c // 2, c % 2
        blocks = J_BLOCKS[j]
        oc = results[c]["out_c"]  # (8, 128, 1024)
        for t, g in enumerate(blocks):
            out[b, P * g : P * (g + 1), :] = oc[t]
    return out


def kernel(x, Wq, bq, Wk, bk, Wv, bv, Wo, bo):
    from concourse.bass_utils import run_bass_kernel_spmd

    nc = build_nc()
    in_maps = make_in_maps(x, Wq, bq, Wk, bk, Wv, bv, Wo, bo)
    res = run_bass_kernel_spmd(nc, in_maps, core_ids=list(range(8)))
    return assemble_out(res.results)

